# revision 1
# baseline (speedup 1.0000x reference)
"""AnalyticalPointNetLK forward on 8 Trainium2 NeuronCores.

Pure data parallel: batch element b -> core b. Everything (PointNet forward,
argmax, analytical Jacobian, 6x6 inverse, Gauss-Newton iterations, se(3) exp
map) runs on-device in one NEFF.

Self-contained: only needs the container's concourse stack.
"""
import sys
import types

import numpy as np


# ---------------------------------------------------------------- env setup
def _setup_env():
    try:
        import concourse.bass  # noqa: F401
    except ImportError:
        sys.path.insert(0, "/opt/trn_rl_repo")

    # Inject antenv.axon_hooks (missing in this image) so trace=True works.
    try:
        import antenv

        if not hasattr(antenv, "axon_hooks"):
            _m = types.ModuleType("antenv.axon_hooks")
            _m._hook = None
            _m.set_axon_ntff_profile_hook = lambda h: setattr(_m, "_hook", h)
            _m.get_axon_ntff_profile_hook = lambda: _m._hook
            sys.modules["antenv.axon_hooks"] = _m
            antenv.axon_hooks = _m
            try:
                from trn_agent_boot.trn_boot import _ntff_profile_via_ctypes

                h = _ntff_profile_via_ctypes("/opt/axon/libaxon_pjrt.so")
                if h is not None:
                    _m.set_axon_ntff_profile_hook(h)
            except Exception:
                pass
    except ImportError:
        pass

    # Split multi-wait exit Drain (this walrus rejects >1 sem wait on CTRL).
    from concourse.tile import TileContext
    from concourse.vector_clock import ScopedClock

    if not getattr(TileContext, "_drain_split_installed", False):

        def _patched(self, tick_clock, wait_clock):
            nc = self.nc
            drain_inst = nc.sync.drain()
            wait_clock.add_sem_waits(
                drain_inst.ins, ScopedClock({None: tick_clock.global_clock})
            )
            si = drain_inst.ins.sync_info
            if si is not None and si.on_wait and len(si.on_wait) > 1:
                waits = list(si.on_wait)
                si.on_wait = waits[:1]
                for w in waits[1:]:
                    extra = nc.sync.drain()
                    esi = extra.ins.sync_info
                    if esi is None:
                        import bass_rust

                        extra.ins.sync_info = bass_rust.SyncInfo(
                            on_wait=[w], on_update=[]
                        )
                    else:
                        esi.on_wait = [w]
            nc.all_engine_barrier()
            assert self.sems is not None
            popped = nc._tile_sem_poison_stack.pop()
            assert popped is self._sem_poison
            nc.clear_and_free_semaphores(list(self.sems.allocated().values()))
            nc.all_engine_barrier()

        TileContext._drain_and_barrier = _patched
        TileContext._drain_split_installed = True


_setup_env()

import concourse.bass as bass  # noqa: E402
import concourse.mybir as mybir  # noqa: E402
from concourse.tile import TileContext  # noqa: E402
from concourse.bass_utils import run_bass_kernel_spmd  # noqa: E402

F32 = mybir.dt.float32
F32R = mybir.dt.float32r
I16 = mybir.dt.int16
ALU = mybir.AluOpType
ACTF = mybir.ActivationFunctionType
AX = mybir.AxisListType

N, K, C1, C2 = 1024, 1024, 64, 128
NCH = 8
EPS = 1e-5
NEG_INF = -3.0e38

# exp-map series (Horner, highest degree first): s1=sin t/t, s2=(1-cos t)/t^2,
# s3=(t-sin t)/t^3 as series in t2=t^2, up to t^12
SER = np.array(
    [
        [1 / 6227020800, 1 / 87178291200, 1 / 1307674368000],
        [-1 / 39916800, -1 / 479001600, -1 / 6227020800],
        [1 / 362880, 1 / 3628800, 1 / 39916800],
        [-1 / 5040, -1 / 40320, -1 / 362880],
        [1 / 120, 1 / 720, 1 / 5040],
        [-1 / 6, -1 / 24, -1 / 120],
        [1.0, 0.5, 1 / 6],
    ],
    np.float32,
)

# ------------------------------------------------------------- consts blob
OFF_IDENT = 0
OFF_IOTA = OFF_IDENT + 128 * 128
OFF_SER = OFF_IOTA + 1024
OFF_S = OFF_SER + 21          # S0|S1|S2 skew generators, [3 rows x 9]
OFF_I3 = OFF_S + 27           # [3 x 3]
OFF_I34 = OFF_I3 + 9          # [3 x 4]
OFF_E4 = OFF_I34 + 12         # [1 x 4]
OFF_I4 = OFF_E4 + 4           # [4 x 4]
OFF_E16 = OFF_I4 + 16         # [16 x 3]
OFF_ONES = OFF_E16 + 48       # [6 x 8] ones
OFF_E6 = OFF_ONES + 48        # [6 x 36]: cE6[:, 6i:6i+6] = row-i-ones matrix
OFF_I6 = OFF_E6 + 216         # [6 x 6]
OFF_ONE1K = OFF_I6 + 36       # [1024] ones
OFF_ZERO1K = OFF_ONE1K + 1024  # [1024] zeros
CONST_LEN = OFF_ZERO1K + 1024


def _build_consts():
    c = np.zeros(CONST_LEN, np.float32)
    c[OFF_IDENT:OFF_IDENT + 128 * 128] = np.eye(128, dtype=np.float32).ravel()
    c[OFF_IOTA:OFF_IOTA + 1024] = np.arange(1024, dtype=np.float32)
    c[OFF_SER:OFF_SER + 21] = SER.ravel()
    S0 = np.array([[0, 0, 0], [0, 0, -1], [0, 1, 0]], np.float32)
    S1 = np.array([[0, 0, 1], [0, 0, 0], [-1, 0, 0]], np.float32)
    S2 = np.array([[0, -1, 0], [1, 0, 0], [0, 0, 0]], np.float32)
    c[OFF_S:OFF_S + 27] = np.concatenate([S0, S1, S2], axis=1).ravel()
    c[OFF_I3:OFF_I3 + 9] = np.eye(3, dtype=np.float32).ravel()
    c[OFF_I34:OFF_I34 + 12] = np.eye(3, 4, dtype=np.float32).ravel()
    c[OFF_E4:OFF_E4 + 4] = np.array([0, 0, 0, 1], np.float32)
    c[OFF_I4:OFF_I4 + 16] = np.eye(4, dtype=np.float32).ravel()
    c[OFF_E16:OFF_E16 + 48] = np.eye(16, 3, dtype=np.float32).ravel()
    c[OFF_ONES:OFF_ONES + 48] = 1.0
    e6 = np.zeros((6, 36), np.float32)
    for i in range(6):
        e6[i, 6 * i:6 * i + 6] = 1.0
    c[OFF_E6:OFF_E6 + 216] = e6.ravel()
    c[OFF_I6:OFF_I6 + 36] = np.eye(6, dtype=np.float32).ravel()
    c[OFF_ONE1K:OFF_ONE1K + 1024] = 1.0
    return c


CONSTS = _build_consts()



def _split_multi_waits(nc):
    """walrus (this build) accepts at most one sem wait per instruction on
    several opcode classes. Split any instruction with >1 waits by inserting
    same-engine nops, each carrying one wait, immediately before it."""
    import bass_rust
    import concourse.mybir as _mb

    def _make_nop(engine):
        h = nc.engines[engine]
        inst = h.nop(nofuse=True)
        # nop() appended to the current bb; detach it
        for f in nc.m.functions:
            for bb in f.blocks:
                lst = bb.instructions
                if lst and lst[-1] is inst.ins:
                    lst.pop()
                    return inst.ins
        raise RuntimeError("nop not found for detach")

    for f in nc.m.functions:
        for bb in f.blocks:
            lst = bb.instructions
            out = []
            changed = False
            for inst in list(lst):
                si = inst.sync_info
                if si is not None and si.on_wait and len(si.on_wait) > 1:
                    waits = list(si.on_wait)
                    for w in waits[:-1]:
                        nop = _make_nop(inst.engine)
                        nop.sync_info = bass_rust.SyncInfo(
                            on_wait=[w], on_update=[])
                        out.append(nop)
                    si.on_wait = [waits[-1]]
                    changed = True
                out.append(inst)
            if changed:
                lst.clear()
                lst.extend(out)


def build_kernel(maxiter: int, use_f32r: bool = True, reduce_ttr: bool = True,
                 debug: bool = False):
    nc = bass.Bass("TRN2", target_bir_lowering=False, debug=False,
                   num_devices=8)

    def din(name, shape, dtype=F32):
        return nc.dram_tensor(name, shape, dtype, kind="ExternalInput").ap()

    p0d = din("p0", [N, 3])
    p1d = din("p1", [N, 3])
    W1d = din("W1", [C1, 3])
    W2Td = din("W2T", [C1, C2])
    W3Td = din("W3T", [C2, K])
    W3nd = din("W3", [K, C2])
    prmd = din("prm", [128, 50])
    constsd = din("consts", [CONST_LEN])
    outd = nc.dram_tensor("out", [K], F32, kind="ExternalOutput").ap()
    dbg = {}
    dxs_only = debug == 2
    if debug:
        dbg["dbg_dxs"] = nc.dram_tensor("dbg_dxs", [1, 64], F32,
                                        kind="ExternalOutput").ap()
    if debug and not dxs_only:
        for nm, shp in [("dbg_m0", [128, 8]), ("dbg_idx", [128, 8]),
                        ("dbg_f0", [128, 8]), ("dbg_J", [128, 48]),
                        ("dbg_Hinv", [6, 6]), ("dbg_g", [4, 4]),
                        ("dbg_x2", [C2, N]), ("dbg_G0", [128, 48])]:
            dbg[nm] = nc.dram_tensor(nm, shp, F32, kind="ExternalOutput").ap()

    with TileContext(nc) as tc:
        with (
            tc.tile_pool(name="per", bufs=1) as per,
            tc.tile_pool(name="pbig", bufs=3, space="PSUM") as pbig,
            tc.tile_pool(name="psm", bufs=2, space="PSUM") as psm,
            tc.tile_pool(name="bbp", bufs=2) as bbp,
            tc.tile_pool(name="dramp", bufs=1, space="DRAM") as dramp,
        ):
            def T(shape, tag, dtype=F32):
                return per.tile(shape, dtype, tag=tag, name=tag)

            def big_psum():
                return pbig.tile([128, 1024], F32, tag="big", name="bigp")

            def small_psum():
                return psm.tile([128, 64], F32, tag="small", name="smallp")

            def heat():
                # dummy matmul keeping the PE HAM warm through the serial
                # expmap tail (PE re-throttles after ~3.4us of idle)
                hp = pbig.tile([128, 512], F32, tag="big", name="heatp")
                nc.tensor.matmul(hp[0:128, 0:512], W3T[:, 0:128],
                                 x2s[:, 0:512], skip_group_check=True)


            def mmr(out, lhsT, rhs, **kw):
                if use_f32r:
                    nc.tensor.matmul(out, lhsT.bitcast(F32R),
                                     rhs.bitcast(F32R), **kw)
                else:
                    nc.tensor.matmul(out, lhsT, rhs, **kw)

            def rr(ap):
                # producers feeding an fp32r matmul must round to fp32r
                return ap.bitcast(F32R) if use_f32r else ap

            def cfill(dst, ones: bool, f32r: bool = False):
                # memset replacement (this walrus rejects InstMemset):
                # broadcast-DMA a constant vector from the consts blob
                p, fsz = dst.shape[0], 1
                for d in dst.shape[1:]:
                    fsz *= d
                assert fsz <= 1024
                off = OFF_ONE1K if ones else OFF_ZERO1K
                srcap = bass.AP(tensor=constsd.tensor,
                                offset=constsd.offset + off,
                                ap=[[0, p], [1, fsz]])
                if f32r and use_f32r:
                    srcap = srcap.bitcast(F32R)
                    dst = rr(dst)
                nc.sync.dma_start(
                    out=dst.rearrange(
                        " ".join(f"d{i}" for i in range(len(dst.shape)))
                        + " -> d0 ("
                        + " ".join(f"d{i}" for i in range(1, len(dst.shape)))
                        + ")") if len(dst.shape) > 2 else dst,
                    in_=srcap)

            # ---------------- const + input DMAs
            def cdma(shape, tag, off, length):
                t = T(shape, tag)
                nc.sync.dma_start(
                    out=t,
                    in_=constsd[off:off + length].rearrange(
                        "(p f) -> p f", p=shape[0]))
                return t

            ident = cdma([128, 128], "ident", OFF_IDENT, 128 * 128)
            ciota = cdma([1, 1024], "ciota", OFF_IOTA, 1024)
            cser = cdma([1, 21], "cser", OFF_SER, 21)
            cS = cdma([3, 9], "cS", OFF_S, 27)
            cI3 = cdma([3, 3], "cI3", OFF_I3, 9)
            cI34 = cdma([3, 4], "cI34", OFF_I34, 12)
            cE4 = cdma([1, 4], "cE4", OFF_E4, 4)
            cI4 = cdma([4, 4], "cI4", OFF_I4, 16)
            cE16 = cdma([16, 3], "cE16", OFF_E16, 48)
            cones = cdma([6, 8], "cones", OFF_ONES, 48)
            cE6 = cdma([6, 36], "cE6", OFF_E6, 216)
            cI6 = cdma([6, 6], "cI6", OFF_I6, 36)

            prm = T([128, 50], "prm")
            nc.sync.dma_start(out=prm, in_=prmd[:, :])
            W1sb = T([C1, 3], "W1sb")
            nc.sync.dma_start(out=W1sb, in_=W1d[:, :])
            W2T = T([C1, C2], "W2T")
            nc.sync.dma_start(out=rr(W2T), in_=W2Td[:, :].bitcast(F32R)
                              if use_f32r else W2Td[:, :])
            p0c4 = T([128, 32], "p0c")
            cfill(p0c4, ones=True)
            nc.sync.dma_start(
                out=p0c4.rearrange("p (k d) -> p k d", d=4)[:, :, 0:3],
                in_=p0d.rearrange("(p k) d -> p k d", p=128))
            p1c = T([128, 32], "p1c")
            cfill(p1c, ones=True)
            nc.sync.dma_start(
                out=p1c.rearrange("p (k d) -> p k d", d=4)[:, :, 0:3],
                in_=p1d.rearrange("(p k) d -> p k d", p=128))
            W3T = T([C2, K], "W3T")
            nc.sync.dma_start(out=rr(W3T), in_=W3Td[:, :].bitcast(F32R)
                              if use_f32r else W3Td[:, :])
            W3n = T([128, 1024], "W3n")
            w3n_src = bass.AP(tensor=W3nd.tensor, offset=W3nd.offset,
                              ap=[[128, 128], [16384, 8], [1, 128]])
            nc.sync.dma_start(
                out=W3n.rearrange("p (c d) -> p c d", d=128), in_=w3n_src)

            # ---------------- param prep: a = gamma*rsqrt(rv+eps),
            # bb = a*(b-rm)+beta  (rsqrt: sqrt + recip + 2 Newton steps)
            def bn_fold(gam, bet, rm, rv, bias, pshape, tagp):
                t = T(pshape, tagp + "_t")
                nc.vector.tensor_scalar_add(t, rv, EPS)
                s = T(pshape, tagp + "_s")
                nc.scalar.sqrt(s, t)
                y = T(pshape, tagp + "_y")
                nc.vector.reciprocal(y, s)
                z = T(pshape, tagp + "_z")
                for _ in range(2):
                    nc.vector.tensor_mul(z, y, y)
                    nc.vector.tensor_mul(z, z, t)
                    nc.vector.tensor_scalar(z, z, -0.5, 1.5, ALU.mult, ALU.add)
                    nc.vector.tensor_mul(y, y, z)
                a = T(pshape, tagp + "_a")
                nc.vector.tensor_mul(a, gam, y)
                bb = T(pshape, tagp + "_bb")
                nc.vector.tensor_sub(bb, bias, rm)
                nc.vector.tensor_mul(bb, bb, a)
                nc.vector.tensor_add(bb, bb, bet)
                return a, bb

            ablk, bbblk = bn_fold(prm[:, 0:10], prm[:, 10:20],
                                  prm[:, 20:30], prm[:, 30:40],
                                  prm[:, 40:50], [128, 10], "bn")
            a1, bb1 = ablk[0:C1, 0:1], bbblk[0:C1, 0:1]
            a2, bb2 = ablk[0:C2, 1:2], bbblk[0:C2, 1:2]
            a3, bb3 = ablk[:, 2:10], bbblk[:, 2:10]

            # ---------------- weight prep
            W1s = T([C1, 3], "W1s")
            nc.vector.tensor_scalar_mul(W1s, W1sb, a1)
            W1s4 = T([C1, 4], "W1s4")
            nc.vector.tensor_copy(W1s4[:, 0:3], W1s)
            nc.vector.tensor_copy(W1s4[:, 3:4], bb1)
            W1aug = T([4, C1], "W1aug")
            tp = small_psum()
            nc.tensor.transpose(tp[0:4, 0:C1], W1s4, ident[0:C1, 0:C1])
            nc.scalar.copy(rr(W1aug), tp[0:4, 0:C1])

            W2aug = T([C1 + 1, C2], "W2aug")
            a2dram = dramp.tile([C2], F32, tag="a2d", name="a2d")
            nc.sync.dma_start(out=a2dram, in_=a2)
            a2bc = T([C1, C2], "a2bc")
            a2bc_src = bass.AP(tensor=a2dram.tensor, offset=a2dram.offset,
                               ap=[[0, C1], [1, C2]])
            nc.sync.dma_start(out=a2bc, in_=a2bc_src)
            nc.vector.tensor_mul(rr(W2aug[0:C1, :]), W2T, a2bc)
            tp4 = psm.tile([128, 128], F32, tag="small", name="smallp")
            nc.tensor.transpose(tp4[0:1, 0:C2], bb2, ident[0:C2, 0:C2])
            nc.scalar.copy(rr(W2aug[C1:C1 + 1, :]), tp4[0:1, 0:C2])

            # ---------------- p0 / p1 transposed homogeneous
            p0hex = T([16, N], "p0hex")
            p1hex = T([4, N], "p1hex")
            cfill(p0hex, ones=False, f32r=True)
            for (srct, dsttile) in ((p0c4, p0hex), (p1c, p1hex)):
                tpp = big_psum()
                for c in range(8):
                    nc.tensor.transpose(
                        tpp[0:4, c * 128:(c + 1) * 128],
                        srct[:, c * 4:(c + 1) * 4], ident)
                nc.vector.tensor_copy(rr(dsttile[0:4, :]), tpp[0:4, 0:N])

            iotabc = T([128, 1024], "iotabc")
            iot_src = bass.AP(tensor=constsd.tensor,
                              offset=constsd.offset + OFF_IOTA,
                              ap=[[0, 128], [1, 1024]])
            nc.sync.dma_start(out=iotabc, in_=iot_src)

            # ---------------- shared tiles
            x1s = T([C1 + 1, N], "x1s")
            cfill(x1s[C1:C1 + 1, :], ones=True, f32r=True)
            x2s = T([C2, N], "x2s")
            scratch = T([128, 1024], "scratch")

            def fwd12(lhsT1, phex):
                u1 = big_psum()
                mmr(u1[0:C1, 0:512], lhsT1, phex[0:4, 0:512])
                mmr(u1[0:C1, 512:1024], lhsT1, phex[0:4, 512:1024])
                nc.scalar.activation(rr(x1s[0:C1, 0:512]), u1[0:C1, 0:512],
                                     ACTF.Relu)
                nc.scalar.activation(rr(x1s[0:C1, 512:1024]),
                                     u1[0:C1, 512:1024], ACTF.Relu)
                u2 = big_psum()
                mmr(u2[0:C2, 0:512], W2aug, x1s[:, 0:512])
                mmr(u2[0:C2, 512:1024], W2aug, x1s[:, 512:1024])
                nc.scalar.activation(rr(x2s[:, 0:512]), u2[0:C2, 0:512],
                                     ACTF.Relu)
                nc.scalar.activation(rr(x2s[:, 512:1024]),
                                     u2[0:C2, 512:1024], ACTF.Relu)

            def l3_chunks(m_out, idx_out):
                for c in range(NCH):
                    ps = big_psum()
                    lh = W3T[:, c * 128:(c + 1) * 128]
                    mmr(ps[0:C2, 0:512], lh, x2s[:, 0:512])
                    mmr(ps[0:C2, 512:1024], lh, x2s[:, 512:1024])
                    nc.vector.tensor_reduce(
                        m_out[:, c:c + 1], ps[:, 0:1024], axis=AX.X,
                        op=ALU.max)
                    if idx_out is not None:
                        nc.vector.scalar_tensor_tensor(
                            out=scratch, in0=ps[:, 0:1024],
                            scalar=m_out[:, c:c + 1], in1=iotabc,
                            op0=ALU.is_ge, op1=ALU.mult,
                            accum_out=idx_out[:, c:c + 1])

            # ---------------- setup forward on p0 (masks + argmax)
            fwd12(W1aug, p0hex)

            m1f = T([C1, N], "m1f")
            nc.vector.tensor_scalar(m1f, x1s[0:C1, :], 0.0, None, ALU.is_gt)
            m2f = T([C2, N], "m2f")
            nc.vector.tensor_scalar(m2f, x2s, 0.0, None, ALU.is_gt)

            m0 = T([128, 8], "m0")
            idxf = T([128, 8], "idxf")
            l3_chunks(m0, idxf)

            f0 = T([128, 8], "f0")
            nc.vector.tensor_mul(f0, m0, a3)
            nc.vector.tensor_add(f0, f0, bb3)
            nc.vector.tensor_scalar_max(f0, f0, 0.0)
            dm3 = T([128, 8], "dm3")
            nc.vector.tensor_scalar(dm3, f0, 0.0, None, ALU.is_gt)
            nc.vector.tensor_mul(dm3, dm3, a3)

            # ---------------- iteration-0 forward early: its L3 compute
            # hides the jacobian chain's DMA phases (table write + gathers)
            m = T([128, 8], "m_it")
            fwd12(W1aug, p1hex)
            l3_chunks(m, None)

            # ---------------- gather via indirect DMA from a DRAM table
            # table row n = [t2mT_0[n,:] | t2mT_1[n,:] | t2mT_2[n,:] | p0[n] |
            # pad] ; gathered per k-chunk with row indices = argmax idx
            import concourse.tile as _tile_mod

            idxu32 = T([128, 8], "idxu32", mybir.dt.uint32)
            nc.vector.tensor_copy(idxu32, idxf)

            # m2f transposed: m2fT_sb[:, 128*nu:128*(nu+1)] = m2f chunk nu ^T
            m2fT = T([128, 1024], "m2fT")
            for nu in range(8):
                tpm = psm.tile([128, 128], F32, tag="small", name="smallp")
                nc.tensor.transpose(tpm, m2f[:, 128 * nu:128 * (nu + 1)],
                                    ident)
                nc.scalar.copy(m2fT[:, 128 * nu:128 * (nu + 1)], tpm)

            t1m = []
            for i in range(3):
                t = T([C1, N], f"t1m{i}")
                nc.vector.tensor_scalar_mul(rr(t), m1f, W1s[:, i:i + 1])
                t1m.append(t)

            CATW = 388
            tcat = T([128, 8 * CATW], "tcat")
            tcat3 = tcat.rearrange("p (nu w) -> p nu w", w=CATW)
            # t2mT chunks: psum[128n, 512] holds nu-batch of 4 for one i
            for i in range(3):
                for half in range(2):
                    psb = pbig.tile([128, 512], F32, tag="big", name="ttp")
                    for q in range(4):
                        nu = half * 4 + q
                        mmr(psb[:, 128 * q:128 * (q + 1)],
                            t1m[i][:, 128 * nu:128 * (nu + 1)],
                            W2aug[0:C1, :])
                    nc.vector.tensor_tensor(
                        out=tcat3[:, 4 * half:4 * half + 4,
                                  128 * i:128 * (i + 1)],
                        in0=psb.rearrange("p (q d) -> p q d", d=128),
                        in1=m2fT.rearrange("p (nu d) -> p nu d", d=128)
                        [:, 4 * half:4 * half + 4, :],
                        op=ALU.mult)
            for nu in range(8):
                nc.scalar.copy(tcat3[:, nu, 384:388],
                               p0c4[:, 4 * nu:4 * nu + 4])

            tcatd = nc.dram_tensor("tcatd", [N, CATW], F32).ap()
            wr_inst = nc.sync.dma_start(
                out=tcatd.rearrange("(nu p) w -> p nu w", p=128), in_=tcat3)

            fjgq = T([128, 24], "fjgq")
            pgs = T([128, 48], "pgs")
            for c in range(NCH):
                gc = bbp.tile([128, CATW], F32, tag="gc", name="gc")
                gi = nc.gpsimd.indirect_dma_start(
                    out=gc[:, :], out_offset=None, in_=tcatd[:, :],
                    in_offset=bass.IndirectOffsetOnAxis(
                        ap=idxu32[:, c:c + 1], axis=0))
                _tile_mod.add_dep_helper(
                    gi.ins, wr_inst.ins, reason="gather waits table write")
                for i in range(3):
                    nc.vector.scalar_tensor_tensor(
                        out=scratch[:, 0:128],
                        in0=gc[:, 128 * i:128 * (i + 1)], scalar=1.0,
                        op0=ALU.mult, in1=W3n[:, 128 * c:128 * (c + 1)],
                        op1=ALU.mult,
                        accum_out=fjgq[:, 3 * c + i:3 * c + i + 1])
                nc.vector.tensor_copy(pgs[:, 6 * c:6 * c + 3],
                                      gc[:, 384:387])
                nc.vector.tensor_copy(pgs[:, 6 * c + 3:6 * c + 6],
                                      gc[:, 384:387])

            fjgs = T([128, 48], "fjgs")
            Jt = T([128, 48], "Jt")
            for c in range(NCH):
                nc.vector.tensor_scalar_mul(
                    fjgs[:, 6 * c:6 * c + 3], fjgq[:, 3 * c:3 * c + 3],
                    dm3[:, c:c + 1])
                nc.vector.tensor_copy(fjgs[:, 6 * c + 3:6 * c + 6],
                                      fjgs[:, 6 * c:6 * c + 3])
            for c in range(NCH):
                # J[:,0:3] = (F2,F0,F1)*(Y,Z,X) - (F1,F2,F0)*(Z,X,Y)
                nc.vector.tensor_mul(Jt[:, 6 * c:6 * c + 3],
                                     fjgs[:, 6 * c + 2:6 * c + 5],
                                     pgs[:, 6 * c + 1:6 * c + 4])
                nc.vector.tensor_mul(scratch[:, 0:3],
                                     fjgs[:, 6 * c + 1:6 * c + 4],
                                     pgs[:, 6 * c + 2:6 * c + 5])
                nc.vector.tensor_sub(Jt[:, 6 * c:6 * c + 3],
                                     Jt[:, 6 * c:6 * c + 3],
                                     scratch[:, 0:3])
                nc.vector.tensor_scalar_mul(Jt[:, 6 * c + 3:6 * c + 6],
                                            fjgs[:, 6 * c:6 * c + 3], -1.0)

            Hp = psm.tile([6, 6], F32, tag="small", name="smallp")
            for c in range(NCH):
                nc.tensor.matmul(Hp, Jt[:, 6 * c:6 * c + 6],
                                 Jt[:, 6 * c:6 * c + 6],
                                 start=(c == 0), stop=(c == NCH - 1))

            # ---------------- 6x6 inverse (unpivoted Gauss-Jordan, SPD)
            M1 = T([6, 12], "M1")
            M2 = T([6, 12], "M2")
            rb6 = T([6, 1], "rb6")
            prow6 = T([6, 12], "prow6")
            ncol = T([6, 1], "ncol")
            nc.scalar.copy(M1[:, 0:6], Hp)
            nc.vector.tensor_copy(M1[:, 6:12], cI6)
            cur, nxt = M1, M2
            for i in range(6):
                pr6 = psm.tile([6, 12], F32, tag="small", name="smallp")
                nc.tensor.matmul(pr6, cE6[:, 6 * i:6 * i + 6], cur)
                nc.vector.reciprocal(rb6, pr6[:, i:i + 1])
                nc.vector.tensor_scalar_mul(prow6, pr6, rb6)
                nc.vector.tensor_scalar_mul(ncol, cur[:, i:i + 1], -1.0)
                nc.vector.tensor_add(ncol, ncol, cI6[:, i:i + 1])
                nc.vector.scalar_tensor_tensor(
                    out=nxt, in0=prow6, scalar=ncol, op0=ALU.mult,
                    in1=cur, op1=ALU.add)
                cur, nxt = nxt, cur
            Hinv = cur[:, 6:12]

            if debug and not dxs_only:
                nc.sync.dma_start(out=dbg["dbg_m0"], in_=m0)
                nc.sync.dma_start(out=dbg["dbg_idx"], in_=idxf)
                nc.sync.dma_start(out=dbg["dbg_f0"], in_=f0)
                nc.sync.dma_start(out=dbg["dbg_J"], in_=Jt)
                nc.sync.dma_start(out=dbg["dbg_Hinv"], in_=Hinv)
                nc.sync.dma_start(out=dbg["dbg_x2"], in_=x2s)
                nc.sync.dma_start(out=dbg["dbg_G0"], in_=fjgs)

            # ---------------- GN iterations
            g_sb = T([4, 4], "g_sb")
            nc.vector.tensor_copy(g_sb, cI4)
            gTsb = T([4, 4], "gTsb")
            W1g = T([4, C1], "W1g")
            f = T([128, 8], "f_it")
            r = T([128, 8], "r_it")
            ysb = T([6, 1], "ysb")
            dxrow = T([1, 6], "dxrow")
            t2s = T([1, 1], "t2s")
            sacc = T([1, 3], "sacc")
            prow8 = T([1, 8], "prow8")
            pb3 = T([3, 8], "pb3")
            nb3 = T([3, 8], "nb3")
            Wm = T([3, 3], "Wm")
            Qm = T([3, 3], "Qm")
            T1 = T([3, 3], "T1m")
            RT4 = T([3, 4], "RT4")
            VT = T([3, 3], "VTm")
            vcol = T([3, 1], "vcol")
            pv4 = T([1, 4], "pv4")
            ET = T([4, 4], "ETm")
            dxs_t = T([1, 64], "dxs_t") if debug else None
            if debug:
                cfill(dxs_t, ones=False)
            cfill(RT4[:, 3:4], ones=False)
            cfill(pv4[:, 3:4], ones=True)
            cfill(prow8, ones=False)

            for it in range(maxiter):
                if it > 0:
                    wp = small_psum()
                    nc.tensor.matmul(wp[0:4, 0:C1], g_sb, W1aug)
                    nc.scalar.copy(rr(W1g), wp[0:4, 0:C1])
                    fwd12(W1g, p1hex)
                    l3_chunks(m, None)
                nc.vector.tensor_mul(f, m, a3)
                nc.vector.tensor_add(f, f, bb3)
                nc.vector.tensor_scalar_max(f, f, 0.0)
                nc.vector.tensor_sub(r, f, f0)
                if it == maxiter - 1:
                    break
                yp = psm.tile([6, 1], F32, tag="small", name="smallp")
                for c in range(NCH):
                    nc.tensor.matmul(yp, Jt[:, 6 * c:6 * c + 6],
                                     r[:, c:c + 1],
                                     start=(c == 0), stop=(c == NCH - 1))
                nc.scalar.copy(ysb, yp)
                heat()
                dxp = psm.tile([1, 6], F32, tag="small", name="smallp")
                nc.tensor.matmul(dxp, ysb, Hinv)
                nc.scalar.copy(dxrow, dxp)
                if debug:
                    nc.vector.tensor_copy(dxs_t[:, 6 * it:6 * it + 6], dxrow)
                # exp map: t2 = |w|^2; s1,s2,s3 by Horner in t2
                nc.vector.tensor_mul(scratch[0:1, 0:3], dxrow[:, 0:3],
                                     dxrow[:, 0:3])
                nc.vector.tensor_reduce(t2s, scratch[0:1, 0:3], axis=AX.X,
                                        op=ALU.add)
                nc.scalar.copy(sacc, cser[:, 0:3])
                for j in range(1, 7):
                    nc.vector.scalar_tensor_tensor(
                        out=sacc, in0=sacc, scalar=t2s, op0=ALU.mult,
                        in1=cser[:, 3 * j:3 * j + 3], op1=ALU.add)
                nc.scalar.copy(prow8[:, 0:1], t2s)
                nc.scalar.copy(prow8[:, 1:4], sacc)
                nc.scalar.copy(prow8[:, 4:7], dxrow[:, 0:3])
                pb3p = psm.tile([3, 8], F32, tag="small", name="smallp")
                nc.tensor.matmul(pb3p, cones[0:1, 0:3], prow8)
                nc.scalar.copy(pb3, pb3p)
                heat()
                nc.scalar.mul(nb3, pb3, -1.0)
                wwp = psm.tile([3, 3], F32, tag="small", name="smallp")
                nc.tensor.matmul(wwp, dxrow[:, 0:3], dxrow[:, 0:3])
                # W = x*S0 + y*S1 + z*S2 ; Q = t2*I - wwT = -W^2
                nc.vector.tensor_scalar_mul(Wm, cS[:, 0:3], pb3[:, 4:5])
                nc.vector.scalar_tensor_tensor(
                    out=Wm, in0=cS[:, 3:6], scalar=pb3[:, 5:6],
                    op0=ALU.mult, in1=Wm, op1=ALU.add)
                nc.vector.scalar_tensor_tensor(
                    out=Wm, in0=cS[:, 6:9], scalar=pb3[:, 6:7],
                    op0=ALU.mult, in1=Wm, op1=ALU.add)
                nc.vector.scalar_tensor_tensor(
                    out=Qm, in0=cI3, scalar=pb3[:, 0:1], op0=ALU.mult,
                    in1=wwp, op1=ALU.subtract)
                # R^T = I - s1 W - s2 Q ; V^T = I - s2 W - s3 Q
                nc.vector.scalar_tensor_tensor(
                    out=T1, in0=Wm, scalar=nb3[:, 1:2], op0=ALU.mult,
                    in1=cI3, op1=ALU.add)
                nc.vector.scalar_tensor_tensor(
                    out=RT4[:, 0:3], in0=Qm, scalar=nb3[:, 2:3],
                    op0=ALU.mult, in1=T1, op1=ALU.add)
                nc.vector.scalar_tensor_tensor(
                    out=T1, in0=Wm, scalar=nb3[:, 2:3], op0=ALU.mult,
                    in1=cI3, op1=ALU.add)
                nc.vector.scalar_tensor_tensor(
                    out=VT, in0=Qm, scalar=nb3[:, 3:4], op0=ALU.mult,
                    in1=T1, op1=ALU.add)
                vcp = psm.tile([3, 1], F32, tag="small", name="smallp")
                nc.tensor.transpose(vcp, dxrow[:, 3:6], ident[0:1, 0:1])
                nc.scalar.copy(vcol, vcp)
                pTp = psm.tile([1, 3], F32, tag="small", name="smallp")
                nc.tensor.matmul(pTp, vcol, VT)
                nc.scalar.copy(pv4[:, 0:3], pTp)
                heat()
                etp = psm.tile([4, 4], F32, tag="small", name="smallp")
                nc.tensor.matmul(etp, cI34, RT4, start=True, stop=False)
                nc.tensor.matmul(etp, cE4, pv4, start=False, stop=True)
                nc.scalar.copy(ET, etp)
                gtp = psm.tile([4, 4], F32, tag="small", name="smallp")
                nc.tensor.matmul(gtp, g_sb, ET)
                nc.scalar.copy(gTsb, gtp)
                heat()
                gp2 = psm.tile([4, 4], F32, tag="small", name="smallp")
                nc.tensor.transpose(gp2, gTsb, ident[0:4, 0:4])
                nc.scalar.copy(g_sb, gp2)
                if debug and not dxs_only and it == 0:
                    nc.sync.dma_start(out=dbg["dbg_g"], in_=g_sb)
            if debug:
                nc.sync.dma_start(out=dbg["dbg_dxs"], in_=dxs_t)

            nc.sync.dma_start(
                out=outd.rearrange("(c p) -> p c", p=128), in_=r)

    _split_multi_waits(nc)
    return nc


# ---------------------------------------------------------------- host side
def _pack_params(inputs):
    # cols: [0:10] gamma(l1,l2,l3k*8), [10:20] beta, [20:30] rm, [30:40] rv,
    # [40:50] conv-bias; layer3 vectors in k-chunk layout
    prm = np.zeros((128, 50), np.float32)
    for g, grp in enumerate([("gamma1", "gamma2", "gamma3"),
                             ("beta1", "beta2", "beta3"),
                             ("rm1", "rm2", "rm3"),
                             ("rv1", "rv2", "rv3"),
                             ("b1", "b2", "b3")]):
        base = 10 * g
        prm[:C1, base + 0] = np.asarray(inputs[grp[0]], np.float32)
        prm[:C2, base + 1] = np.asarray(inputs[grp[1]], np.float32)
        v = np.asarray(inputs[grp[2]], np.float32).reshape(8, 128)
        prm[:, base + 2:base + 10] = v.T
    return prm


def make_in_maps(inputs):
    B = int(np.asarray(inputs["p0"]).shape[0])
    prm = _pack_params(inputs)
    W2T = np.ascontiguousarray(np.asarray(inputs["W2"], np.float32).T)
    W3n = np.ascontiguousarray(np.asarray(inputs["W3"], np.float32))
    W3T = np.ascontiguousarray(W3n.T)
    p0 = np.asarray(inputs["p0"], np.float32)
    p1 = np.asarray(inputs["p1"], np.float32)
    W1 = np.ascontiguousarray(np.asarray(inputs["W1"], np.float32))
    return [
        {
            "p0": np.ascontiguousarray(p0[b]),
            "p1": np.ascontiguousarray(p1[b]),
            "W1": W1,
            "W2T": W2T,
            "W3T": W3T,
            "W3": W3n,
            "prm": prm,
            "consts": CONSTS,
        }
        for b in range(B)
    ]


_NC_CACHE = {}
TRACE = False
LAST_RESULT = None
USE_F32R = False
REDUCE_TTR = True


def kernel(**inputs):
    global LAST_RESULT
    maxiter = int(np.asarray(inputs["maxiter"]))
    B = int(np.asarray(inputs["p0"]).shape[0])
    if maxiter <= 0:
        return np.zeros((B, K), np.float32)

    key = (maxiter, USE_F32R, REDUCE_TTR)
    if key not in _NC_CACHE:
        _NC_CACHE[key] = build_kernel(maxiter, use_f32r=USE_F32R,
                                      reduce_ttr=REDUCE_TTR)
    nc = _NC_CACHE[key]
    in_maps = make_in_maps(inputs)
    res = run_bass_kernel_spmd(nc, in_maps, core_ids=list(range(B)),
                               trace=TRACE)
    LAST_RESULT = res
    return np.stack([res.results[b]["out"] for b in range(B)], axis=0)



# revision 29
# speedup vs baseline: 1.2053x; 1.2053x over previous
"""AnalyticalPointNetLK forward on 8 Trainium2 NeuronCores.

Pure data parallel: batch element b -> core b. Everything (PointNet forward,
argmax, analytical Jacobian, 6x6 inverse, Gauss-Newton iterations, se(3) exp
map) runs on-device in one NEFF.

Mixed precision: setup + GN iterations 0..2 run fp32 matmuls (the GN
dynamics amplify early noise ~10x for this regime); iterations 3+ run fp32r
(single-pass PE, 2x faster). Dense short f32r "heat" matmuls keep the PE's
HAM clock-gate at K=8/8 (2.4 GHz) through the serial exp-map tail.

Self-contained: only needs the container's concourse stack.
"""
import sys
import types

import numpy as np


# ---------------------------------------------------------------- env setup
def _setup_env():
    try:
        import concourse.bass  # noqa: F401
    except ImportError:
        sys.path.insert(0, "/opt/trn_rl_repo")

    # Inject antenv.axon_hooks (missing in this image) so trace=True works.
    try:
        import antenv

        if not hasattr(antenv, "axon_hooks"):
            _m = types.ModuleType("antenv.axon_hooks")
            _m._hook = None
            _m.set_axon_ntff_profile_hook = lambda h: setattr(_m, "_hook", h)
            _m.get_axon_ntff_profile_hook = lambda: _m._hook
            sys.modules["antenv.axon_hooks"] = _m
            antenv.axon_hooks = _m
            try:
                from trn_agent_boot.trn_boot import _ntff_profile_via_ctypes

                h = _ntff_profile_via_ctypes("/opt/axon/libaxon_pjrt.so")
                if h is not None:
                    _m.set_axon_ntff_profile_hook(h)
            except Exception:
                pass
    except ImportError:
        pass

    # Split multi-wait exit Drain (this walrus rejects >1 sem wait on CTRL).
    from concourse.tile import TileContext
    from concourse.vector_clock import ScopedClock

    if not getattr(TileContext, "_drain_split_installed", False):

        def _patched(self, tick_clock, wait_clock):
            nc = self.nc
            drain_inst = nc.sync.drain()
            wait_clock.add_sem_waits(
                drain_inst.ins, ScopedClock({None: tick_clock.global_clock})
            )
            si = drain_inst.ins.sync_info
            if si is not None and si.on_wait and len(si.on_wait) > 1:
                waits = list(si.on_wait)
                si.on_wait = waits[:1]
                for w in waits[1:]:
                    extra = nc.sync.drain()
                    esi = extra.ins.sync_info
                    if esi is None:
                        import bass_rust

                        extra.ins.sync_info = bass_rust.SyncInfo(
                            on_wait=[w], on_update=[]
                        )
                    else:
                        esi.on_wait = [w]
            nc.all_engine_barrier()
            assert self.sems is not None
            popped = nc._tile_sem_poison_stack.pop()
            assert popped is self._sem_poison
            nc.clear_and_free_semaphores(list(self.sems.allocated().values()))
            nc.all_engine_barrier()

        TileContext._drain_and_barrier = _patched
        TileContext._drain_split_installed = True


_setup_env()

import concourse.bass as bass  # noqa: E402
import concourse.mybir as mybir  # noqa: E402
from concourse.tile import TileContext  # noqa: E402
from concourse.bass_utils import run_bass_kernel_spmd  # noqa: E402

F32 = mybir.dt.float32
F32R = mybir.dt.float32r
ALU = mybir.AluOpType
ACTF = mybir.ActivationFunctionType
AX = mybir.AxisListType

N, K, C1, C2 = 1024, 1024, 64, 128
NCH = 8
EPS = 1e-5
NEG_INF = -3.0e38

# exp-map series (Horner, highest degree first): s1=sin t/t, s2=(1-cos t)/t^2,
# s3=(t-sin t)/t^3 as series in t2=t^2, up to t^12
SER = np.array(
    [
        [1 / 6227020800, 1 / 87178291200, 1 / 1307674368000],
        [-1 / 39916800, -1 / 479001600, -1 / 6227020800],
        [1 / 362880, 1 / 3628800, 1 / 39916800],
        [-1 / 5040, -1 / 40320, -1 / 362880],
        [1 / 120, 1 / 720, 1 / 5040],
        [-1 / 6, -1 / 24, -1 / 120],
        [1.0, 0.5, 1 / 6],
    ],
    np.float32,
)

# ------------------------------------------------------------- consts blob
OFF_IDENT = 0
OFF_IOTA = OFF_IDENT + 128 * 128
OFF_SER = OFF_IOTA + 1024
OFF_S = OFF_SER + 21          # S0|S1|S2 skew generators, [3 rows x 9]
OFF_I3 = OFF_S + 27           # [3 x 3]
OFF_I34 = OFF_I3 + 9          # [3 x 4]
OFF_E4 = OFF_I34 + 12         # [1 x 4]
OFF_I4 = OFF_E4 + 4           # [4 x 4]
OFF_E16 = OFF_I4 + 16         # [16 x 3]
OFF_ONES = OFF_E16 + 48       # [6 x 8] ones
OFF_E6 = OFF_ONES + 48        # [6 x 36]: cE6[:, 6i:6i+6] = row-i-ones matrix
OFF_I6 = OFF_E6 + 216         # [6 x 6]
OFF_ONE1K = OFF_I6 + 36       # [1024] ones
OFF_ZERO1K = OFF_ONE1K + 1024  # [1024] zeros
CONST_LEN = OFF_ZERO1K + 1024


def _build_consts():
    c = np.zeros(CONST_LEN, np.float32)
    c[OFF_IDENT:OFF_IDENT + 128 * 128] = np.eye(128, dtype=np.float32).ravel()
    c[OFF_IOTA:OFF_IOTA + 1024] = np.arange(1024, dtype=np.float32)
    c[OFF_SER:OFF_SER + 21] = SER.ravel()
    S0 = np.array([[0, 0, 0], [0, 0, -1], [0, 1, 0]], np.float32)
    S1 = np.array([[0, 0, 1], [0, 0, 0], [-1, 0, 0]], np.float32)
    S2 = np.array([[0, -1, 0], [1, 0, 0], [0, 0, 0]], np.float32)
    c[OFF_S:OFF_S + 27] = np.concatenate([S0, S1, S2], axis=1).ravel()
    c[OFF_I3:OFF_I3 + 9] = np.eye(3, dtype=np.float32).ravel()
    c[OFF_I34:OFF_I34 + 12] = np.eye(3, 4, dtype=np.float32).ravel()
    c[OFF_E4:OFF_E4 + 4] = np.array([0, 0, 0, 1], np.float32)
    c[OFF_I4:OFF_I4 + 16] = np.eye(4, dtype=np.float32).ravel()
    c[OFF_E16:OFF_E16 + 48] = np.eye(16, 3, dtype=np.float32).ravel()
    c[OFF_ONES:OFF_ONES + 48] = 1.0
    e6 = np.zeros((6, 36), np.float32)
    for i in range(6):
        e6[i, 6 * i:6 * i + 6] = 1.0
    c[OFF_E6:OFF_E6 + 216] = e6.ravel()
    c[OFF_I6:OFF_I6 + 36] = np.eye(6, dtype=np.float32).ravel()
    c[OFF_ONE1K:OFF_ONE1K + 1024] = 1.0
    return c


CONSTS = _build_consts()


def _split_multi_waits(nc):
    """walrus (this build) accepts at most one sem wait per instruction on
    several opcode classes. Split any instruction with >1 waits by inserting
    same-engine nops, each carrying one wait, immediately before it."""
    import bass_rust
    import concourse.mybir as _mb

    def _make_nop(engine):
        h = nc.engines[engine]
        inst = h.nop(nofuse=True)
        # nop() appended to the current bb; detach it
        for f in nc.m.functions:
            for bb in f.blocks:
                lst = bb.instructions
                if lst and lst[-1] is inst.ins:
                    lst.pop()
                    return inst.ins
        raise RuntimeError("nop not found for detach")

    for f in nc.m.functions:
        for bb in f.blocks:
            lst = bb.instructions
            out = []
            changed = False
            for inst in list(lst):
                si = inst.sync_info
                if si is not None and si.on_wait and len(si.on_wait) > 1:
                    waits = list(si.on_wait)
                    for w in waits[:-1]:
                        nop = _make_nop(inst.engine)
                        nop.sync_info = bass_rust.SyncInfo(
                            on_wait=[w], on_update=[])
                        out.append(nop)
                    si.on_wait = [waits[-1]]
                    changed = True
                out.append(inst)
            if changed:
                lst.clear()
                lst.extend(out)


def build_kernel(maxiter: int, use_f32r: bool = True, f32r_from: int = 3,
                 debug: bool = False):
    nc = bass.Bass("TRN2", target_bir_lowering=False, debug=False,
                   num_devices=8)

    def din(name, shape, dtype=F32):
        return nc.dram_tensor(name, shape, dtype, kind="ExternalInput").ap()

    p0d = din("p0", [N, 3])
    p1d = din("p1", [N, 3])
    W1d = din("W1", [C1, 3])
    W2Td = din("W2T", [C1, C2])
    W3Td = din("W3T", [C2, K])
    W3nd = din("W3", [K, C2])
    prmd = din("prm", [128, 50])
    constsd = din("consts", [CONST_LEN])
    outd = nc.dram_tensor("out", [K], F32, kind="ExternalOutput").ap()
    dbg = {}
    if debug:
        dbg["dbg_dxs"] = nc.dram_tensor("dbg_dxs", [1, 64], F32,
                                        kind="ExternalOutput").ap()
        for nm, shp in [("dbg_m0", [128, 8]), ("dbg_idx", [128, 8]),
                        ("dbg_f0", [128, 8]), ("dbg_J", [128, 48]),
                        ("dbg_Hinv", [6, 6])]:
            dbg[nm] = nc.dram_tensor(nm, shp, F32, kind="ExternalOutput").ap()

    assert f32r_from >= 1  # iteration 0's forward shares fp32-tagged tiles

    def itr(it):
        # f32r for this iteration's forward?
        return use_f32r and it >= f32r_from

    with TileContext(nc) as tc:
        with (
            tc.tile_pool(name="per", bufs=1) as per,
            tc.tile_pool(name="pbig", bufs=3, space="PSUM") as pbig,
            tc.tile_pool(name="psm", bufs=2, space="PSUM") as psm,
            tc.tile_pool(name="bbp", bufs=2) as bbp,
            tc.tile_pool(name="dramp", bufs=1, space="DRAM") as dramp,
        ):
            def T(shape, tag, dtype=F32):
                return per.tile(shape, dtype, tag=tag, name=tag)

            def big_psum():
                return pbig.tile([128, 1024], F32, tag="big", name="bigp")

            def small_psum():
                return psm.tile([128, 64], F32, tag="small", name="smallp")

            def mm(out, lhsT, rhs, r, **kw):
                if r:
                    nc.tensor.matmul(out, lhsT.bitcast(F32R),
                                     rhs.bitcast(F32R), **kw)
                else:
                    nc.tensor.matmul(out, lhsT, rhs, **kw)

            def rr(ap, r=True):
                # producers feeding an fp32r matmul must be tagged fp32r
                return ap.bitcast(F32R) if r else ap

            def cfill(dst, ones: bool, f32r: bool = False):
                # memset replacement (this walrus rejects InstMemset):
                # broadcast-DMA a constant vector from the consts blob
                p, fsz = dst.shape[0], 1
                for d in dst.shape[1:]:
                    fsz *= d
                assert fsz <= 1024
                off = OFF_ONE1K if ones else OFF_ZERO1K
                srcap = bass.AP(tensor=constsd.tensor,
                                offset=constsd.offset + off,
                                ap=[[0, p], [1, fsz]])
                if f32r:
                    srcap = srcap.bitcast(F32R)
                    dst = rr(dst)
                nc.sync.dma_start(
                    out=dst.rearrange(
                        " ".join(f"d{i}" for i in range(len(dst.shape)))
                        + " -> d0 ("
                        + " ".join(f"d{i}" for i in range(1, len(dst.shape)))
                        + ")") if len(dst.shape) > 2 else dst,
                    in_=srcap)

            # ---------------- const + input DMAs
            def cdma(shape, tag, off, length):
                t = T(shape, tag)
                nc.sync.dma_start(
                    out=t,
                    in_=constsd[off:off + length].rearrange(
                        "(p f) -> p f", p=shape[0]))
                return t

            ident = cdma([128, 128], "ident", OFF_IDENT, 128 * 128)
            iotabc = T([128, 1024], "iotabc")
            iot_src = bass.AP(tensor=constsd.tensor,
                              offset=constsd.offset + OFF_IOTA,
                              ap=[[0, 128], [1, 1024]])
            nc.sync.dma_start(out=iotabc, in_=iot_src)
            cser = cdma([1, 21], "cser", OFF_SER, 21)
            cS = cdma([3, 9], "cS", OFF_S, 27)
            cI3 = cdma([3, 3], "cI3", OFF_I3, 9)
            cI34 = cdma([3, 4], "cI34", OFF_I34, 12)
            cE4 = cdma([1, 4], "cE4", OFF_E4, 4)
            cI4 = cdma([4, 4], "cI4", OFF_I4, 16)
            cones = cdma([6, 8], "cones", OFF_ONES, 48)
            cE6 = cdma([6, 36], "cE6", OFF_E6, 216)
            cI6 = cdma([6, 6], "cI6", OFF_I6, 36)

            # dummy matmuls keeping the PE HAM warm through the serial
            # expmap tail (PE re-throttles after ~3.4us of idle)
            def heat(n=1, cols=512):
                for _ in range(n):
                    hp = pbig.tile([128, 512], F32, tag="big", name="heatp")
                    nc.tensor.matmul(hp[0:128, 0:cols], W3T[:, 0:128],
                                     x2s[:, 0:cols], skip_group_check=True)

            prm = T([128, 50], "prm")
            nc.sync.dma_start(out=prm, in_=prmd[:, :])
            W1sb = T([C1, 3], "W1sb")
            nc.sync.dma_start(out=W1sb, in_=W1d[:, :])
            W2T = T([C1, C2], "W2T")
            nc.sync.dma_start(out=W2T, in_=W2Td[:, :])
            p0c4 = T([128, 32], "p0c")
            cfill(p0c4, ones=True)
            nc.sync.dma_start(
                out=p0c4.rearrange("p (k d) -> p k d", d=4)[:, :, 0:3],
                in_=p0d.rearrange("(p k) d -> p k d", p=128))
            p1c = T([128, 32], "p1c")
            cfill(p1c, ones=True)
            nc.sync.dma_start(
                out=p1c.rearrange("p (k d) -> p k d", d=4)[:, :, 0:3],
                in_=p1d.rearrange("(p k) d -> p k d", p=128))
            W3T = T([C2, K], "W3T")
            nc.sync.dma_start(out=W3T, in_=W3Td[:, :])
            W3TR = T([C2, K], "W3TR")
            nc.sync.dma_start(out=rr(W3TR), in_=W3Td[:, :].bitcast(F32R))
            W3n = T([128, 1024], "W3n")
            w3n_src = bass.AP(tensor=W3nd.tensor, offset=W3nd.offset,
                              ap=[[128, 128], [16384, 8], [1, 128]])
            nc.sync.dma_start(
                out=W3n.rearrange("p (c d) -> p c d", d=128), in_=w3n_src)

            # ---------------- param prep: a = gamma*rsqrt(rv+eps),
            # bb = a*(b-rm)+beta  (rsqrt: sqrt + recip + 2 Newton steps)
            def bn_fold(gam, bet, rm, rv, bias, pshape, tagp):
                t = T(pshape, tagp + "_t")
                nc.vector.tensor_scalar_add(t, rv, EPS)
                s = T(pshape, tagp + "_s")
                nc.scalar.sqrt(s, t)
                y = T(pshape, tagp + "_y")
                nc.vector.reciprocal(y, s)
                z = T(pshape, tagp + "_z")
                for _ in range(2):
                    nc.vector.tensor_mul(z, y, y)
                    nc.vector.tensor_mul(z, z, t)
                    nc.vector.tensor_scalar(z, z, -0.5, 1.5, ALU.mult, ALU.add)
                    nc.vector.tensor_mul(y, y, z)
                a = T(pshape, tagp + "_a")
                nc.vector.tensor_mul(a, gam, y)
                bb = T(pshape, tagp + "_bb")
                nc.vector.tensor_sub(bb, bias, rm)
                nc.vector.tensor_mul(bb, bb, a)
                nc.vector.tensor_add(bb, bb, bet)
                return a, bb

            ablk, bbblk = bn_fold(prm[:, 0:10], prm[:, 10:20],
                                  prm[:, 20:30], prm[:, 30:40],
                                  prm[:, 40:50], [128, 10], "bn")
            a1, bb1 = ablk[0:C1, 0:1], bbblk[0:C1, 0:1]
            a2, bb2 = ablk[0:C2, 1:2], bbblk[0:C2, 1:2]
            a3, bb3 = ablk[:, 2:10], bbblk[:, 2:10]

            # ---------------- weight prep
            W1s = T([C1, 3], "W1s")
            nc.vector.tensor_scalar_mul(W1s, W1sb, a1)
            W1s4 = T([C1, 4], "W1s4")
            nc.vector.tensor_copy(W1s4[:, 0:3], W1s)
            nc.vector.tensor_copy(W1s4[:, 3:4], bb1)
            W1aug = T([4, C1], "W1aug")
            tp = small_psum()
            nc.tensor.transpose(tp[0:4, 0:C1], W1s4, ident[0:C1, 0:C1])
            nc.scalar.copy(W1aug, tp[0:4, 0:C1])

            W2aug = T([C1 + 1, C2], "W2aug")
            a2dram = dramp.tile([C2], F32, tag="a2d", name="a2d")
            nc.sync.dma_start(out=a2dram, in_=a2)
            a2bc = T([C1, C2], "a2bc")
            a2bc_src = bass.AP(tensor=a2dram.tensor, offset=a2dram.offset,
                               ap=[[0, C1], [1, C2]])
            nc.sync.dma_start(out=a2bc, in_=a2bc_src)
            nc.vector.tensor_mul(W2aug[0:C1, :], W2T, a2bc)
            tp4 = psm.tile([128, 128], F32, tag="small", name="smallp")
            nc.tensor.transpose(tp4[0:1, 0:C2], bb2, ident[0:C2, 0:C2])
            nc.scalar.copy(W2aug[C1:C1 + 1, :], tp4[0:1, 0:C2])
            # f32r twin for late-iteration forwards
            W2augR = T([C1 + 1, C2], "W2augR")
            nc.vector.tensor_copy(rr(W2augR), W2aug)

            # ---------------- p0 / p1 transposed homogeneous
            p0hex = T([16, N], "p0hex")
            p1hex = T([4, N], "p1hex")
            p1hexR = T([4, N], "p1hexR")
            cfill(p0hex, ones=False)
            for (srct, dsttile) in ((p0c4, p0hex), (p1c, p1hex)):
                tpp = big_psum()
                for c in range(8):
                    nc.tensor.transpose(
                        tpp[0:4, c * 128:(c + 1) * 128],
                        srct[:, c * 4:(c + 1) * 4], ident)
                nc.vector.tensor_copy(dsttile[0:4, :], tpp[0:4, 0:N])
                if dsttile is p1hex:
                    nc.vector.tensor_copy(rr(p1hexR[0:4, :]), tpp[0:4, 0:N])

            # ---------------- shared tiles (x = fp32 path, xR = f32r path)
            x1s = T([C1 + 1, N], "x1s")
            cfill(x1s[C1:C1 + 1, :], ones=True)
            x1sR = T([C1 + 1, N], "x1sR")
            cfill(x1sR[C1:C1 + 1, :], ones=True, f32r=True)
            x2s = T([C2, N], "x2s")
            x2sR = T([C2, N], "x2sR")
            scratch = T([128, 1024], "scratch")

            def fwd12(lhsT1, phex, r):
                x1, x2, w2 = (x1sR, x2sR, W2augR) if r else (x1s, x2s, W2aug)
                u1 = big_psum()
                mm(u1[0:C1, 0:512], lhsT1, phex[0:4, 0:512], r)
                mm(u1[0:C1, 512:1024], lhsT1, phex[0:4, 512:1024], r)
                nc.scalar.activation(rr(x1[0:C1, 0:512], r), u1[0:C1, 0:512],
                                     ACTF.Relu)
                nc.scalar.activation(rr(x1[0:C1, 512:1024], r),
                                     u1[0:C1, 512:1024], ACTF.Relu)
                u2 = big_psum()
                mm(u2[0:C2, 0:512], w2, x1[:, 0:512], r)
                mm(u2[0:C2, 512:1024], w2, x1[:, 512:1024], r)
                nc.scalar.activation(rr(x2[:, 0:512], r), u2[0:C2, 0:512],
                                     ACTF.Relu)
                nc.scalar.activation(rr(x2[:, 512:1024], r),
                                     u2[0:C2, 512:1024], ACTF.Relu)

            def l3_chunks(m_out, idx_out, r):
                x2 = x2sR if r else x2s
                w3 = W3TR if r else W3T
                for c in range(NCH):
                    ps = big_psum()
                    lh = w3[:, c * 128:(c + 1) * 128]
                    mm(ps[0:C2, 0:512], lh, x2[:, 0:512], r)
                    mm(ps[0:C2, 512:1024], lh, x2[:, 512:1024], r)
                    nc.vector.tensor_reduce(
                        m_out[:, c:c + 1], ps[:, 0:1024], axis=AX.X,
                        op=ALU.max)
                    if idx_out is not None:
                        nc.vector.scalar_tensor_tensor(
                            out=scratch, in0=ps[:, 0:1024],
                            scalar=m_out[:, c:c + 1], in1=iotabc,
                            op0=ALU.is_ge, op1=ALU.mult,
                            accum_out=idx_out[:, c:c + 1])

            # ---------------- setup forward on p0 (masks + argmax), fp32
            fwd12(W1aug, p0hex, False)

            m1f = T([C1, N], "m1f")
            nc.vector.tensor_scalar(m1f, x1s[0:C1, :], 0.0, None, ALU.is_gt)
            m2f = T([C2, N], "m2f")
            nc.vector.tensor_scalar(m2f, x2s, 0.0, None, ALU.is_gt)

            m0 = T([128, 8], "m0")
            idxf = T([128, 8], "idxf")
            l3_chunks(m0, idxf, False)

            f0 = T([128, 8], "f0")
            nc.vector.tensor_mul(f0, m0, a3)
            nc.vector.tensor_add(f0, f0, bb3)
            nc.vector.tensor_scalar_max(f0, f0, 0.0)
            dm3 = T([128, 8], "dm3")
            nc.vector.tensor_scalar(dm3, f0, 0.0, None, ALU.is_gt)
            nc.vector.tensor_mul(dm3, dm3, a3)

            # ---------------- iteration-0 forward early: its L3 compute
            # hides the jacobian chain's DMA phases (table write + gathers)
            m = T([128, 8], "m_it")
            fwd12(W1aug, p1hex, itr(0))
            l3_chunks(m, None, itr(0))

            # ---------------- jacobian chain -> DRAM gather table
            import concourse.tile as _tile_mod

            idxu32 = T([128, 8], "idxu32", mybir.dt.uint32)
            nc.vector.tensor_copy(idxu32, idxf)

            # m2f transposed: m2fT_sb[:, 128*nu:128*(nu+1)] = m2f chunk nu ^T
            m2fT = T([128, 1024], "m2fT")
            for nu in range(8):
                tpm = psm.tile([128, 128], F32, tag="small", name="smallp")
                nc.tensor.transpose(tpm, m2f[:, 128 * nu:128 * (nu + 1)],
                                    ident)
                nc.scalar.copy(m2fT[:, 128 * nu:128 * (nu + 1)], tpm)

            t1m = []
            for i in range(3):
                t = T([C1, N], f"t1m{i}")
                nc.vector.tensor_scalar_mul(t, m1f, W1s[:, i:i + 1])
                t1m.append(t)

            CATW = 388
            tcat = T([128, 8 * CATW], "tcat")
            tcat3 = tcat.rearrange("p (nu w) -> p nu w", w=CATW)
            # t2mT chunks: psum[128n, 512] holds nu-batch of 4 for one i
            for i in range(3):
                for half in range(2):
                    psb = pbig.tile([128, 512], F32, tag="big", name="ttp")
                    for q in range(4):
                        nu = half * 4 + q
                        mm(psb[:, 128 * q:128 * (q + 1)],
                           t1m[i][:, 128 * nu:128 * (nu + 1)],
                           W2aug[0:C1, :], False)
                    nc.vector.tensor_tensor(
                        out=tcat3[:, 4 * half:4 * half + 4,
                                  128 * i:128 * (i + 1)],
                        in0=psb.rearrange("p (q d) -> p q d", d=128),
                        in1=m2fT.rearrange("p (nu d) -> p nu d", d=128)
                        [:, 4 * half:4 * half + 4, :],
                        op=ALU.mult)
            for nu in range(8):
                nc.scalar.copy(tcat3[:, nu, 384:388],
                               p0c4[:, 4 * nu:4 * nu + 4])

            tcatd = nc.dram_tensor("tcatd", [N, CATW], F32).ap()
            wr_inst = nc.sync.dma_start(
                out=tcatd.rearrange("(nu p) w -> p nu w", p=128), in_=tcat3)

            # ---------------- gather via indirect DMA from the DRAM table
            # table row n = [t2mT_0[n,:] | t2mT_1[n,:] | t2mT_2[n,:] | p0[n] |
            # pad] ; gathered per k-chunk with row indices = argmax idx
            fjgq = T([128, 24], "fjgq")
            pgs = T([128, 48], "pgs")
            for c in range(NCH):
                gc = bbp.tile([128, CATW], F32, tag="gc", name="gc")
                gi = nc.gpsimd.indirect_dma_start(
                    out=gc[:, :], out_offset=None, in_=tcatd[:, :],
                    in_offset=bass.IndirectOffsetOnAxis(
                        ap=idxu32[:, c:c + 1], axis=0))
                _tile_mod.add_dep_helper(
                    gi.ins, wr_inst.ins, reason="gather waits table write")
                for i in range(3):
                    nc.vector.scalar_tensor_tensor(
                        out=scratch[:, 0:128],
                        in0=gc[:, 128 * i:128 * (i + 1)], scalar=1.0,
                        op0=ALU.mult, in1=W3n[:, 128 * c:128 * (c + 1)],
                        op1=ALU.mult,
                        accum_out=fjgq[:, 3 * c + i:3 * c + i + 1])
                nc.vector.tensor_copy(pgs[:, 6 * c:6 * c + 3],
                                      gc[:, 384:387])
                nc.vector.tensor_copy(pgs[:, 6 * c + 3:6 * c + 6],
                                      gc[:, 384:387])

            fjgs = T([128, 48], "fjgs")
            Jt = T([128, 48], "Jt")
            for c in range(NCH):
                nc.vector.tensor_scalar_mul(
                    fjgs[:, 6 * c:6 * c + 3], fjgq[:, 3 * c:3 * c + 3],
                    dm3[:, c:c + 1])
                nc.vector.tensor_copy(fjgs[:, 6 * c + 3:6 * c + 6],
                                      fjgs[:, 6 * c:6 * c + 3])
            for c in range(NCH):
                # J[:,0:3] = (F2,F0,F1)*(Y,Z,X) - (F1,F2,F0)*(Z,X,Y)
                nc.vector.tensor_mul(Jt[:, 6 * c:6 * c + 3],
                                     fjgs[:, 6 * c + 2:6 * c + 5],
                                     pgs[:, 6 * c + 1:6 * c + 4])
                nc.vector.tensor_mul(scratch[:, 0:3],
                                     fjgs[:, 6 * c + 1:6 * c + 4],
                                     pgs[:, 6 * c + 2:6 * c + 5])
                nc.vector.tensor_sub(Jt[:, 6 * c:6 * c + 3],
                                     Jt[:, 6 * c:6 * c + 3],
                                     scratch[:, 0:3])
                nc.vector.tensor_scalar_mul(Jt[:, 6 * c + 3:6 * c + 6],
                                            fjgs[:, 6 * c:6 * c + 3], -1.0)

            Hp = psm.tile([6, 6], F32, tag="small", name="smallp")
            for c in range(NCH):
                nc.tensor.matmul(Hp, Jt[:, 6 * c:6 * c + 6],
                                 Jt[:, 6 * c:6 * c + 6],
                                 start=(c == 0), stop=(c == NCH - 1))

            # ---------------- 6x6 inverse (unpivoted Gauss-Jordan, SPD)
            M1 = T([6, 12], "M1")
            M2 = T([6, 12], "M2")
            rb6 = T([6, 1], "rb6")
            prow6 = T([6, 12], "prow6")
            ncol = T([6, 1], "ncol")
            nc.scalar.copy(M1[:, 0:6], Hp)
            nc.vector.tensor_copy(M1[:, 6:12], cI6)
            cur, nxt = M1, M2
            for i in range(6):
                pr6 = psm.tile([6, 12], F32, tag="small", name="smallp")
                nc.tensor.matmul(pr6, cE6[:, 6 * i:6 * i + 6], cur)
                nc.vector.reciprocal(rb6, pr6[:, i:i + 1])
                nc.vector.tensor_scalar_mul(prow6, pr6, rb6)
                nc.vector.tensor_scalar_mul(ncol, cur[:, i:i + 1], -1.0)
                nc.vector.tensor_add(ncol, ncol, cI6[:, i:i + 1])
                nc.vector.scalar_tensor_tensor(
                    out=nxt, in0=prow6, scalar=ncol, op0=ALU.mult,
                    in1=cur, op1=ALU.add)
                cur, nxt = nxt, cur
            Hinv = cur[:, 6:12]

            if debug:
                nc.sync.dma_start(out=dbg["dbg_m0"], in_=m0)
                nc.sync.dma_start(out=dbg["dbg_idx"], in_=idxf)
                nc.sync.dma_start(out=dbg["dbg_f0"], in_=f0)
                nc.sync.dma_start(out=dbg["dbg_J"], in_=Jt)
                nc.sync.dma_start(out=dbg["dbg_Hinv"], in_=Hinv)

            # ---------------- GN iterations
            g_sb = T([4, 4], "g_sb")
            nc.vector.tensor_copy(g_sb, cI4)
            W1g = T([4, C1], "W1g")
            W1gR = T([4, C1], "W1gR")
            f = T([128, 8], "f_it")
            r = T([128, 8], "r_it")
            ysb = T([6, 1], "ysb")
            dxrow = T([1, 6], "dxrow")
            t2s = T([1, 1], "t2s")
            sacc = T([1, 3], "sacc")
            prow8 = T([1, 8], "prow8")
            pb3 = T([3, 8], "pb3")
            nb3 = T([3, 8], "nb3")
            vcol = T([3, 1], "vcol")
            gTsb = T([4, 4], "gTsb")
            Wm = T([3, 3], "Wm")
            Qm = T([3, 3], "Qm")
            T1 = T([3, 3], "T1m")
            RT4 = T([3, 4], "RT4")
            VT = T([3, 3], "VTm")
            pv4 = T([1, 4], "pv4")
            ET = T([4, 4], "ETm")
            dxs_t = T([1, 64], "dxs_t") if debug else None
            if debug:
                cfill(dxs_t, ones=False)
            cfill(RT4[:, 3:4], ones=False)
            cfill(pv4[:, 3:4], ones=True)
            cfill(prow8, ones=False)

            for it in range(maxiter):
                if it > 0:
                    ri = itr(it)
                    w1 = W1gR if ri else W1g
                    wp = small_psum()
                    nc.tensor.matmul(wp[0:4, 0:C1], g_sb, W1aug)
                    nc.scalar.copy(rr(w1, ri), wp[0:4, 0:C1])
                    fwd12(w1, p1hexR if ri else p1hex, ri)
                    l3_chunks(m, None, ri)
                nc.vector.tensor_mul(f, m, a3)
                nc.vector.tensor_add(f, f, bb3)
                nc.vector.tensor_scalar_max(f, f, 0.0)
                nc.vector.tensor_sub(r, f, f0)
                if it == maxiter - 1:
                    break
                yp = psm.tile([6, 1], F32, tag="small", name="smallp")
                for c in range(NCH):
                    nc.tensor.matmul(yp, Jt[:, 6 * c:6 * c + 6],
                                     r[:, c:c + 1],
                                     start=(c == 0), stop=(c == NCH - 1))
                nc.scalar.copy(ysb, yp)
                heat()
                dxp = psm.tile([1, 6], F32, tag="small", name="smallp")
                nc.tensor.matmul(dxp, ysb, Hinv)
                nc.scalar.copy(dxrow, dxp)
                if debug:
                    nc.vector.tensor_copy(dxs_t[:, 6 * it:6 * it + 6], dxrow)
                # exp map: t2 = |w|^2; s1,s2,s3 by Horner in t2
                nc.vector.tensor_mul(scratch[0:1, 0:3], dxrow[:, 0:3],
                                     dxrow[:, 0:3])
                nc.vector.tensor_reduce(t2s, scratch[0:1, 0:3], axis=AX.X,
                                        op=ALU.add)
                nc.scalar.copy(sacc, cser[:, 0:3])
                for j in range(1, 7):
                    nc.vector.scalar_tensor_tensor(
                        out=sacc, in0=sacc, scalar=t2s, op0=ALU.mult,
                        in1=cser[:, 3 * j:3 * j + 3], op1=ALU.add)
                nc.scalar.copy(prow8[:, 0:1], t2s)
                nc.scalar.copy(prow8[:, 1:4], sacc)
                nc.scalar.copy(prow8[:, 4:7], dxrow[:, 0:3])
                pb3p = psm.tile([3, 8], F32, tag="small", name="smallp")
                nc.tensor.matmul(pb3p, cones[0:1, 0:3], prow8)
                nc.scalar.copy(pb3, pb3p)
                heat()
                nc.scalar.mul(nb3, pb3, -1.0)
                wwp = psm.tile([3, 3], F32, tag="small", name="smallp")
                nc.tensor.matmul(wwp, dxrow[:, 0:3], dxrow[:, 0:3])
                # W = x*S0 + y*S1 + z*S2 ; Q = t2*I - wwT = -W^2
                nc.vector.tensor_scalar_mul(Wm, cS[:, 0:3], pb3[:, 4:5])
                nc.vector.scalar_tensor_tensor(
                    out=Wm, in0=cS[:, 3:6], scalar=pb3[:, 5:6],
                    op0=ALU.mult, in1=Wm, op1=ALU.add)
                nc.vector.scalar_tensor_tensor(
                    out=Wm, in0=cS[:, 6:9], scalar=pb3[:, 6:7],
                    op0=ALU.mult, in1=Wm, op1=ALU.add)
                nc.vector.scalar_tensor_tensor(
                    out=Qm, in0=cI3, scalar=pb3[:, 0:1], op0=ALU.mult,
                    in1=wwp, op1=ALU.subtract)
                # R^T = I - s1 W - s2 Q ; V^T = I - s2 W - s3 Q
                nc.vector.scalar_tensor_tensor(
                    out=T1, in0=Wm, scalar=nb3[:, 1:2], op0=ALU.mult,
                    in1=cI3, op1=ALU.add)
                nc.vector.scalar_tensor_tensor(
                    out=RT4[:, 0:3], in0=Qm, scalar=nb3[:, 2:3],
                    op0=ALU.mult, in1=T1, op1=ALU.add)
                nc.vector.scalar_tensor_tensor(
                    out=T1, in0=Wm, scalar=nb3[:, 2:3], op0=ALU.mult,
                    in1=cI3, op1=ALU.add)
                nc.vector.scalar_tensor_tensor(
                    out=VT, in0=Qm, scalar=nb3[:, 3:4], op0=ALU.mult,
                    in1=T1, op1=ALU.add)
                vcp = psm.tile([3, 1], F32, tag="small", name="smallp")
                nc.tensor.transpose(vcp, dxrow[:, 3:6], ident[0:1, 0:1])
                nc.scalar.copy(vcol, vcp)
                pTp = psm.tile([1, 3], F32, tag="small", name="smallp")
                nc.tensor.matmul(pTp, vcol, VT)
                nc.scalar.copy(pv4[:, 0:3], pTp)
                heat()
                etp = psm.tile([4, 4], F32, tag="small", name="smallp")
                nc.tensor.matmul(etp, cI34, RT4, start=True, stop=False)
                nc.tensor.matmul(etp, cE4, pv4, start=False, stop=True)
                nc.scalar.copy(ET, etp)
                gtp = psm.tile([4, 4], F32, tag="small", name="smallp")
                nc.tensor.matmul(gtp, g_sb, ET)
                nc.scalar.copy(gTsb, gtp)
                heat()
                gp2 = psm.tile([4, 4], F32, tag="small", name="smallp")
                nc.tensor.transpose(gp2, gTsb, ident[0:4, 0:4])
                nc.scalar.copy(g_sb, gp2)
            if debug:
                nc.sync.dma_start(out=dbg["dbg_dxs"], in_=dxs_t)

            nc.sync.dma_start(
                out=outd.rearrange("(c p) -> p c", p=128), in_=r)

    _split_multi_waits(nc)
    return nc


# ---------------------------------------------------------------- host side
def _pack_params(inputs):
    # cols: [0:10] gamma(l1,l2,l3k*8), [10:20] beta, [20:30] rm, [30:40] rv,
    # [40:50] conv-bias; layer3 vectors in k-chunk layout
    prm = np.zeros((128, 50), np.float32)
    for g, grp in enumerate([("gamma1", "gamma2", "gamma3"),
                             ("beta1", "beta2", "beta3"),
                             ("rm1", "rm2", "rm3"),
                             ("rv1", "rv2", "rv3"),
                             ("b1", "b2", "b3")]):
        base = 10 * g
        prm[:C1, base + 0] = np.asarray(inputs[grp[0]], np.float32)
        prm[:C2, base + 1] = np.asarray(inputs[grp[1]], np.float32)
        v = np.asarray(inputs[grp[2]], np.float32).reshape(8, 128)
        prm[:, base + 2:base + 10] = v.T
    return prm


def make_in_maps(inputs):
    B = int(np.asarray(inputs["p0"]).shape[0])
    prm = _pack_params(inputs)
    W2T = np.ascontiguousarray(np.asarray(inputs["W2"], np.float32).T)
    W3n = np.ascontiguousarray(np.asarray(inputs["W3"], np.float32))
    W3T = np.ascontiguousarray(W3n.T)
    p0 = np.asarray(inputs["p0"], np.float32)
    p1 = np.asarray(inputs["p1"], np.float32)
    W1 = np.ascontiguousarray(np.asarray(inputs["W1"], np.float32))
    return [
        {
            "p0": np.ascontiguousarray(p0[b]),
            "p1": np.ascontiguousarray(p1[b]),
            "W1": W1,
            "W2T": W2T,
            "W3T": W3T,
            "W3": W3n,
            "prm": prm,
            "consts": CONSTS,
        }
        for b in range(B)
    ]


_NC_CACHE = {}
TRACE = False
LAST_RESULT = None
USE_F32R = True
F32R_FROM = 3
REDUCE_TTR = True


def kernel(**inputs):
    global LAST_RESULT
    maxiter = int(np.asarray(inputs["maxiter"]))
    B = int(np.asarray(inputs["p0"]).shape[0])
    if maxiter <= 0:
        return np.zeros((B, K), np.float32)

    key = (maxiter, USE_F32R, F32R_FROM)
    if key not in _NC_CACHE:
        _NC_CACHE[key] = build_kernel(maxiter, use_f32r=USE_F32R,
                                      f32r_from=F32R_FROM)
    nc = _NC_CACHE[key]
    in_maps = make_in_maps(inputs)
    res = run_bass_kernel_spmd(nc, in_maps, core_ids=list(range(B)),
                               trace=TRACE)
    LAST_RESULT = res
    return np.stack([res.results[b]["out"] for b in range(B)], axis=0)
